# revision 2
# baseline (speedup 1.0000x reference)
"""Trainium2 Bass kernel for nn_CombinedRotaryEmbedding.

Math: the 32 sequential Givens rotations and the learned rotation_matrix
compose into a single 64x64 matrix M (host-precomputed).  The RoPE stage
  out_top = y1*cos - y2*sin ; out_bot = y1*sin + y2*cos
is rewritten as out = u # COS + w # SIN with
  u = x @ Mbig   (rows = [Y1 | Y2] per head-pair)
  w = x @ Msw    (rows = [-Y2 | Y1])
so no cross-partition data movement is needed on-device.

v2: all HBM I/O in bf16 (halves the DMA roofline to ~45us/core), bf16
matmuls (4x PE rate vs fp32r).  The fp32-PSUM drain is the critical
resource: DVE tensor_tensor from PSUM runs at 1x (no 16-bit packing), so
the drain is split across engines per group of 1024 columns:
  - DVE drain-mul: t1[0:512]   = u1 # cos      (PSUM, 1x, mul fused free)
  - Scalar casts:  ub2 = bf16(u2), wb = bf16(w) (ACT copy, idle engine)
  - DVE bf16 2x:   t1[512:1024] = ub2 # cos ; t2 = wb # sin
  - adds t1+t2 split GpSimd (0:768) / DVE (768:1024)

Sharding: sequence-parallel over 8 cores (1024 positions each).  The host
pre-transposes x to [core][128 partitions = (head%2, d_in)][b, head//2, s]
so the PE can contract over d_in with full 128-partition utilisation, and
inverse-permutes the output.
"""

import numpy as np


def _import_bass():
    try:
        import concourse.bass  # noqa: F401
    except ImportError:
        import sys

        sys.path.insert(0, "/opt/trn_rl_repo")


_import_bass()

import concourse.bass as bass  # noqa: E402
import concourse.mybir as mybir  # noqa: E402
from concourse.tile import TileContext  # noqa: E402
from concourse.vector_clock import ScopedClock  # noqa: E402

import ml_dtypes  # noqa: E402

B, S, NSTATE = 4, 8192, 1024
H, D, NUM_ROT = 16, 64, 32
NCORES = 8
S_SH = S // NCORES  # 1024 positions per core
FREE = B * (H // 2) * S_SH  # 32768 columns per core
CHUNK = 4096  # x-columns per DMA
GROUP = 1024  # x-columns per PSUM group == one (b, head-pair) s-block
ADD_SPLIT = 768  # adds: [0:ADD_SPLIT] on GpSimd, rest on DVE

F32 = mybir.dt.float32
BF16 = mybir.dt.bfloat16
NP_BF16 = ml_dtypes.bfloat16


class _TileContextSplitDrain(TileContext):
    """TileContext whose final drain carries at most one sem wait per
    instruction — the walrus in this container rejects instructions
    with 2+ sync waits ("Too many sync wait commands")."""

    def _drain_and_barrier(self, tick_clock, wait_clock):
        nc = self.nc
        drain_inst = nc.sync.drain()
        wait_clock.add_sem_waits(
            drain_inst.ins, ScopedClock({None: tick_clock.global_clock})
        )
        si = drain_inst.ins.sync_info
        waits = list(si.on_wait or [])
        if len(waits) > 1:
            si.on_wait = [waits[0]]
            for w in waits[1:]:
                n = nc.sync.nop(nofuse=True, hint="drain_wait_split")
                n.ins.sync_info = type(si)(on_update=[], on_wait=[w])
        nc.all_engine_barrier()
        assert self.sems is not None
        popped = nc._tile_sem_poison_stack.pop()
        assert popped is self._sem_poison
        nc.clear_and_free_semaphores(list(self.sems.allocated().values()))
        nc.all_engine_barrier()


def _split_excess_waits(nc, limit=1):
    """Walrus here rejects instructions with >limit sync waits.  Hoist
    excess waits onto same-engine InstNoOps inserted immediately before
    the offending instruction (same engine stream => program order)."""
    n_split = 0
    for fn in nc.m.functions:
        for blk in fn.blocks:
            insts = blk.instructions
            i = 0
            while i < len(insts):
                inst = insts[i]
                si = getattr(inst, "sync_info", None)
                waits = list(si.on_wait) if (si and si.on_wait) else []
                if len(waits) > limit:
                    keep = waits[-limit:]
                    excess = waits[:-limit]
                    si.on_wait = keep
                    for j, w in enumerate(excess):
                        nop = mybir.InstNoOp(
                            name=f"{inst.name}-wsplit{j}",
                            engine=inst.engine,
                            bass_nofuse=True,
                            sync_info=mybir.SyncInfo(on_wait=[w], on_update=[]),
                        )
                        insts.insert(i, nop)
                        i += 1
                        n_split += 1
                i += 1
    return n_split


def compose_rotation(thetas: np.ndarray, rotation_matrix: np.ndarray) -> np.ndarray:
    """Fold the sequential Givens rotations + rotation_matrix into one 64x64."""
    M = np.eye(D, dtype=np.float64)
    th = thetas.astype(np.float64)
    for k in range(NUM_ROT):
        i, j = k % D, (k + 1) % D
        c, s = np.cos(th[k]), np.sin(th[k])
        mi = M[:, i] * c + M[:, j] * s
        mj = -M[:, i] * s + M[:, j] * c
        M[:, i], M[:, j] = mi, mj
    return M @ rotation_matrix.astype(np.float64)


def build_weights(thetas: np.ndarray, rotation_matrix: np.ndarray):
    """Mbig (u = [Y1|Y2]) and Msw (w = [-Y2|Y1]) as [k=128, m=128] bf16."""
    M64 = compose_rotation(thetas, rotation_matrix)
    Mev = M64[:, 0::2]  # y1 columns [64, 32]
    Mod = M64[:, 1::2]  # y2 columns
    Mbig = np.zeros((128, 128), dtype=np.float64)
    Msw = np.zeros((128, 128), dtype=np.float64)
    for hp in (0, 1):
        r = slice(hp * 64, hp * 64 + 64)
        c1 = slice(hp * 32, hp * 32 + 32)
        c2 = slice(64 + hp * 32, 64 + hp * 32 + 32)
        Mbig[r, c1] = Mev
        Mbig[r, c2] = Mod
        Msw[r, c1] = -Mod
        Msw[r, c2] = Mev
    return Mbig.astype(NP_BF16), Msw.astype(NP_BF16)


def build_tables(inv_freq: np.ndarray):
    """Per-core cos/sin tables [128, 1024] each, row p uses inv_freq[p % 32],
    column j is position core_base + j.

    Args are computed in fp32 to match the reference's fp32 `pos * inv_freq`
    rounding; sin/cos mirror the reference's jax lowering when available.
    """
    invf = inv_freq.astype(np.float32)
    try:
        import jax.numpy as jnp

        pos = jnp.arange(S, dtype=jnp.float32)
        sinusoid = pos[:, None] * jnp.asarray(invf)[None, :]  # [S, 32]
        sin_all = np.asarray(jnp.sin(sinusoid))
        cos_all = np.asarray(jnp.cos(sinusoid))
    except Exception:
        args = np.arange(S, dtype=np.float32)[:, None] * invf[None, :]
        sin_all, cos_all = np.sin(args), np.cos(args)

    l = np.arange(128) % 32
    ctabs = np.empty((NCORES, 128, GROUP), dtype=NP_BF16)
    stabs = np.empty((NCORES, 128, GROUP), dtype=NP_BF16)
    for c in range(NCORES):
        sl = slice(c * S_SH, (c + 1) * S_SH)
        ctabs[c] = cos_all[sl].T[l].astype(NP_BF16)  # [128, 1024]
        stabs[c] = sin_all[sl].T[l].astype(NP_BF16)
    return ctabs, stabs


def shard_x(x: np.ndarray) -> np.ndarray:
    """[B,S,1024] -> [core, 128 (hp,d), FREE (b,hi,s)] contiguous bf16."""
    xr = np.ascontiguousarray(x).reshape(B, NCORES, S_SH, H // 2, 2, D)
    xt = xr.transpose(1, 4, 5, 0, 3, 2)  # (core, hp, d, b, hi, sl)
    return np.ascontiguousarray(xt).astype(NP_BF16).reshape(NCORES, 128, FREE)


def unshard_out(o: np.ndarray) -> np.ndarray:
    """[core, 128 (half,hp,l), FREE (b,hi,s)] bf16 -> [B,S,1024] f32."""
    orr = o.astype(np.float32).reshape(NCORES, 2, 2, 32, B, H // 2, S_SH)
    ot = orr.transpose(4, 0, 6, 5, 2, 1, 3)  # (b, core, sl, hi, hp, half, l)
    return np.ascontiguousarray(ot).reshape(B, S, NSTATE)


_NC_CACHE = {}


def _build_nc():
    if "nc" in _NC_CACHE:
        return _NC_CACHE["nc"]
    nc = bass.Bass(trn_type="TRN2")
    x_d = nc.dram_tensor("x", [128, FREE], BF16, kind="ExternalInput")
    mb_d = nc.dram_tensor("mb", [128, 128], BF16, kind="ExternalInput")
    msw_d = nc.dram_tensor("msw", [128, 128], BF16, kind="ExternalInput")
    ctab_d = nc.dram_tensor("ctab", [128, GROUP], BF16, kind="ExternalInput")
    stab_d = nc.dram_tensor("stab", [128, GROUP], BF16, kind="ExternalInput")
    o_d = nc.dram_tensor("o", [128, FREE], BF16, kind="ExternalOutput")

    with _TileContextSplitDrain(nc) as tc:
        with tc.tile_pool(name="const", bufs=1) as cpool, \
             tc.tile_pool(name="xin", bufs=3) as xpool, \
             tc.tile_pool(name="t12", bufs=3) as tpool, \
             tc.tile_pool(name="oout", bufs=2) as opool, \
             tc.tile_pool(name="psum", bufs=2, space="PSUM") as ppool:
            mb = cpool.tile([128, 128], BF16, tag="mb")
            msw = cpool.tile([128, 128], BF16, tag="msw")
            ctab = cpool.tile([128, GROUP], BF16, tag="ctab")
            stab = cpool.tile([128, GROUP], BF16, tag="stab")
            nc.sync.dma_start(out=mb, in_=mb_d.ap())
            nc.sync.dma_start(out=msw, in_=msw_d.ap())
            nc.sync.dma_start(out=ctab, in_=ctab_d.ap())
            nc.sync.dma_start(out=stab, in_=stab_d.ap())

            for ch in range(FREE // CHUNK):
                xt = xpool.tile([128, CHUNK], BF16)
                nc.sync.dma_start(
                    out=xt, in_=x_d.ap()[:, ch * CHUNK : (ch + 1) * CHUNK]
                )
                ot = opool.tile([128, CHUNK], BF16)
                for g in range(CHUNK // GROUP):
                    base = g * GROUP
                    ps = ppool.tile([128, 2 * GROUP], F32)
                    xs1 = xt[:, base : base + 512]
                    xs2 = xt[:, base + 512 : base + 1024]
                    # u = ps[:, 0:1024], w = ps[:, 1024:2048]
                    nc.tensor.matmul(ps[:, 0:512], lhsT=mb, rhs=xs1,
                                     start=True, stop=True)
                    nc.tensor.matmul(ps[:, 512:1024], lhsT=mb, rhs=xs2,
                                     start=True, stop=True)
                    nc.tensor.matmul(ps[:, 1024:1536], lhsT=msw, rhs=xs1,
                                     start=True, stop=True)
                    nc.tensor.matmul(ps[:, 1536:2048], lhsT=msw, rhs=xs2,
                                     start=True, stop=True)

                    t1 = tpool.tile([128, GROUP], BF16)
                    t2 = tpool.tile([128, GROUP], BF16)
                    ub2 = tpool.tile([128, 512], BF16)
                    wb = tpool.tile([128, GROUP], BF16)
                    # DVE drain-mul of u1 (PSUM 1x)
                    nc.vector.tensor_mul(
                        out=t1[:, 0:512], in0=ps[:, 0:512], in1=ctab[:, 0:512]
                    )
                    # Scalar engine drains u2 and w (ACT copy+cast)
                    nc.scalar.copy(out=ub2, in_=ps[:, 512:1024])
                    nc.scalar.copy(out=wb, in_=ps[:, 1024:2048])
                    # bf16 2x muls on DVE
                    nc.vector.tensor_mul(
                        out=t1[:, 512:1024], in0=ub2, in1=ctab[:, 512:1024]
                    )
                    nc.vector.tensor_mul(out=t2, in0=wb, in1=stab)
                    # final add split GpSimd / DVE
                    ov = ot[:, base : base + GROUP]
                    nc.gpsimd.tensor_add(
                        out=ov[:, 0:ADD_SPLIT],
                        in0=t1[:, 0:ADD_SPLIT],
                        in1=t2[:, 0:ADD_SPLIT],
                    )
                    nc.vector.tensor_add(
                        out=ov[:, ADD_SPLIT:GROUP],
                        in0=t1[:, ADD_SPLIT:GROUP],
                        in1=t2[:, ADD_SPLIT:GROUP],
                    )
                nc.sync.dma_start(
                    out=o_d.ap()[:, ch * CHUNK : (ch + 1) * CHUNK], in_=ot
                )
    _split_excess_waits(nc)
    _NC_CACHE["nc"] = nc
    return nc


def kernel(x, thetas, rotation_matrix, inv_freq, _trace=False):
    from concourse.bass_utils import run_bass_kernel_spmd

    x = np.asarray(x, dtype=np.float32)
    thetas = np.asarray(thetas, dtype=np.float32)
    rotation_matrix = np.asarray(rotation_matrix, dtype=np.float32)
    inv_freq = np.asarray(inv_freq, dtype=np.float32)

    Mbig, Msw = build_weights(thetas, rotation_matrix)
    ctabs, stabs = build_tables(inv_freq)
    xs = shard_x(x)

    nc = _build_nc()
    in_maps = [
        {"x": xs[c], "mb": Mbig, "msw": Msw, "ctab": ctabs[c], "stab": stabs[c]}
        for c in range(NCORES)
    ]
    res = run_bass_kernel_spmd(
        nc, in_maps, core_ids=list(range(NCORES)), trace=_trace
    )
    o = np.stack([res.results[c]["o"] for c in range(NCORES)])
    out = unshard_out(o)
    if _trace:
        return out, res
    return out


# revision 5
# speedup vs baseline: 1.0776x; 1.0776x over previous
"""Trainium2 Bass kernel for nn_CombinedRotaryEmbedding.

Math: the 32 sequential Givens rotations and the learned rotation_matrix
compose into a single 64x64 matrix M (host-precomputed).  The RoPE stage
  out_top = y1*cos - y2*sin ; out_bot = y1*sin + y2*cos
is rewritten as out = u # COS + w # SIN with
  u = x @ Mbig   (rows = [Y1 | Y2] per head-pair)
  w = x @ Msw    (rows = [-Y2 | Y1])
so no cross-partition data movement is needed on-device.

v2: all HBM I/O in bf16 (halves the DMA roofline to ~45us/core), bf16
matmuls (4x PE rate vs fp32r).  The fp32-PSUM drain is the critical
resource: DVE tensor_tensor from PSUM runs at 1x (no 16-bit packing), so
the drain is split across engines per group of 1024 columns:
  - DVE drain-mul: t1[0:512]   = u1 # cos      (PSUM, 1x, mul fused free)
  - Scalar casts:  ub2 = bf16(u2), wb = bf16(w) (ACT copy, idle engine)
  - DVE bf16 2x:   t1[512:1024] = ub2 # cos ; t2 = wb # sin
  - adds t1+t2 split GpSimd (0:768) / DVE (768:1024)

Sharding: sequence-parallel over 8 cores (1024 positions each).  The host
pre-transposes x to [core][128 partitions = (head%2, d_in)][b, head//2, s]
so the PE can contract over d_in with full 128-partition utilisation, and
inverse-permutes the output.
"""

import numpy as np


def _import_bass():
    try:
        import concourse.bass  # noqa: F401
    except ImportError:
        import sys

        sys.path.insert(0, "/opt/trn_rl_repo")


_import_bass()

import concourse.bass as bass  # noqa: E402
import concourse.mybir as mybir  # noqa: E402
from concourse.tile import TileContext  # noqa: E402
from concourse.vector_clock import ScopedClock  # noqa: E402

import ml_dtypes  # noqa: E402

B, S, NSTATE = 4, 8192, 1024
H, D, NUM_ROT = 16, 64, 32
NCORES = 8
S_SH = S // NCORES  # 1024 positions per core
FREE = B * (H // 2) * S_SH  # 32768 columns per core
CHUNK = 8192  # x-columns per DMA (16KB bf16 per partition line)
GROUP = 1024  # x-columns per PSUM group == one (b, head-pair) s-block
DVE_ADD_EVERY = 4  # every 4th group's final add runs on DVE, rest on GpSimd

F32 = mybir.dt.float32
BF16 = mybir.dt.bfloat16
NP_BF16 = ml_dtypes.bfloat16


class _TileContextSplitDrain(TileContext):
    """TileContext whose final drain carries at most one sem wait per
    instruction — the walrus in this container rejects instructions
    with 2+ sync waits ("Too many sync wait commands")."""

    def _drain_and_barrier(self, tick_clock, wait_clock):
        nc = self.nc
        drain_inst = nc.sync.drain()
        wait_clock.add_sem_waits(
            drain_inst.ins, ScopedClock({None: tick_clock.global_clock})
        )
        si = drain_inst.ins.sync_info
        waits = list(si.on_wait or [])
        if len(waits) > 1:
            si.on_wait = [waits[0]]
            for w in waits[1:]:
                n = nc.sync.nop(nofuse=True, hint="drain_wait_split")
                n.ins.sync_info = type(si)(on_update=[], on_wait=[w])
        nc.all_engine_barrier()
        assert self.sems is not None
        popped = nc._tile_sem_poison_stack.pop()
        assert popped is self._sem_poison
        nc.clear_and_free_semaphores(list(self.sems.allocated().values()))
        nc.all_engine_barrier()


def _split_excess_waits(nc, limit=1):
    """Walrus here rejects instructions with >limit sync waits.  Hoist
    excess waits onto same-engine InstNoOps inserted immediately before
    the offending instruction (same engine stream => program order)."""
    n_split = 0
    for fn in nc.m.functions:
        for blk in fn.blocks:
            insts = blk.instructions
            i = 0
            while i < len(insts):
                inst = insts[i]
                si = getattr(inst, "sync_info", None)
                waits = list(si.on_wait) if (si and si.on_wait) else []
                if len(waits) > limit:
                    keep = waits[-limit:]
                    excess = waits[:-limit]
                    si.on_wait = keep
                    for j, w in enumerate(excess):
                        nop = mybir.InstNoOp(
                            name=f"{inst.name}-wsplit{j}",
                            engine=inst.engine,
                            bass_nofuse=True,
                            sync_info=mybir.SyncInfo(on_wait=[w], on_update=[]),
                        )
                        insts.insert(i, nop)
                        i += 1
                        n_split += 1
                i += 1
    return n_split


def compose_rotation(thetas: np.ndarray, rotation_matrix: np.ndarray) -> np.ndarray:
    """Fold the sequential Givens rotations + rotation_matrix into one 64x64."""
    M = np.eye(D, dtype=np.float64)
    th = thetas.astype(np.float64)
    for k in range(NUM_ROT):
        i, j = k % D, (k + 1) % D
        c, s = np.cos(th[k]), np.sin(th[k])
        mi = M[:, i] * c + M[:, j] * s
        mj = -M[:, i] * s + M[:, j] * c
        M[:, i], M[:, j] = mi, mj
    return M @ rotation_matrix.astype(np.float64)


def build_weights(thetas: np.ndarray, rotation_matrix: np.ndarray):
    """Mbig (u = [Y1|Y2]) and Msw (w = [-Y2|Y1]) as [k=128, m=128] bf16."""
    M64 = compose_rotation(thetas, rotation_matrix)
    Mev = M64[:, 0::2]  # y1 columns [64, 32]
    Mod = M64[:, 1::2]  # y2 columns
    Mbig = np.zeros((128, 128), dtype=np.float64)
    Msw = np.zeros((128, 128), dtype=np.float64)
    for hp in (0, 1):
        r = slice(hp * 64, hp * 64 + 64)
        c1 = slice(hp * 32, hp * 32 + 32)
        c2 = slice(64 + hp * 32, 64 + hp * 32 + 32)
        Mbig[r, c1] = Mev
        Mbig[r, c2] = Mod
        Msw[r, c1] = -Mod
        Msw[r, c2] = Mev
    return Mbig.astype(NP_BF16), Msw.astype(NP_BF16)


def build_tables(inv_freq: np.ndarray):
    """Per-core cos/sin tables [128, 1024] each, row p uses inv_freq[p % 32],
    column j is position core_base + j.

    Args are computed in fp32 to match the reference's fp32 `pos * inv_freq`
    rounding; sin/cos mirror the reference's jax lowering when available.
    """
    invf = inv_freq.astype(np.float32)
    try:
        import jax.numpy as jnp

        pos = jnp.arange(S, dtype=jnp.float32)
        sinusoid = pos[:, None] * jnp.asarray(invf)[None, :]  # [S, 32]
        sin_all = np.asarray(jnp.sin(sinusoid))
        cos_all = np.asarray(jnp.cos(sinusoid))
    except Exception:
        args = np.arange(S, dtype=np.float32)[:, None] * invf[None, :]
        sin_all, cos_all = np.sin(args), np.cos(args)

    l = np.arange(128) % 32
    ctabs = np.empty((NCORES, 128, GROUP), dtype=NP_BF16)
    stabs = np.empty((NCORES, 128, GROUP), dtype=NP_BF16)
    for c in range(NCORES):
        sl = slice(c * S_SH, (c + 1) * S_SH)
        ctabs[c] = cos_all[sl].T[l].astype(NP_BF16)  # [128, 1024]
        stabs[c] = sin_all[sl].T[l].astype(NP_BF16)
    return ctabs, stabs


def shard_x(x: np.ndarray) -> np.ndarray:
    """[B,S,1024] -> [core, 128 (hp,d), FREE (b,hi,s)] contiguous bf16."""
    xr = np.ascontiguousarray(x).reshape(B, NCORES, S_SH, H // 2, 2, D)
    xt = xr.transpose(1, 4, 5, 0, 3, 2)  # (core, hp, d, b, hi, sl)
    return np.ascontiguousarray(xt).astype(NP_BF16).reshape(NCORES, 128, FREE)


def unshard_out(o: np.ndarray) -> np.ndarray:
    """[core, 128 (half,hp,l), FREE (b,hi,s)] bf16 -> [B,S,1024] f32."""
    orr = o.astype(np.float32).reshape(NCORES, 2, 2, 32, B, H // 2, S_SH)
    ot = orr.transpose(4, 0, 6, 5, 2, 1, 3)  # (b, core, sl, hi, hp, half, l)
    return np.ascontiguousarray(ot).reshape(B, S, NSTATE)


_NC_CACHE = {}


def _build_nc():
    if "nc" in _NC_CACHE:
        return _NC_CACHE["nc"]
    nc = bass.Bass(trn_type="TRN2")
    x_d = nc.dram_tensor("x", [128, FREE], BF16, kind="ExternalInput")
    mb_d = nc.dram_tensor("mb", [128, 128], BF16, kind="ExternalInput")
    msw_d = nc.dram_tensor("msw", [128, 128], BF16, kind="ExternalInput")
    ctab_d = nc.dram_tensor("ctab", [128, GROUP], BF16, kind="ExternalInput")
    stab_d = nc.dram_tensor("stab", [128, GROUP], BF16, kind="ExternalInput")
    o_d = nc.dram_tensor("o", [128, FREE], BF16, kind="ExternalOutput")

    with _TileContextSplitDrain(nc) as tc:
        with tc.tile_pool(name="const", bufs=1) as cpool, \
             tc.tile_pool(name="xin", bufs=3) as xpool, \
             tc.tile_pool(name="t12", bufs=3) as tpool, \
             tc.tile_pool(name="oout", bufs=2) as opool, \
             tc.tile_pool(name="psum", bufs=2, space="PSUM") as ppool:
            mb = cpool.tile([128, 128], BF16, tag="mb")
            msw = cpool.tile([128, 128], BF16, tag="msw")
            ctab = cpool.tile([128, GROUP], BF16, tag="ctab")
            stab = cpool.tile([128, GROUP], BF16, tag="stab")
            nc.sync.dma_start(out=mb, in_=mb_d.ap())
            nc.sync.dma_start(out=msw, in_=msw_d.ap())
            nc.sync.dma_start(out=ctab, in_=ctab_d.ap())
            nc.sync.dma_start(out=stab, in_=stab_d.ap())

            for ch in range(FREE // CHUNK):
                xt = xpool.tile([128, CHUNK], BF16)
                nc.sync.dma_start(
                    out=xt, in_=x_d.ap()[:, ch * CHUNK : (ch + 1) * CHUNK]
                )
                ot = opool.tile([128, CHUNK], BF16)
                for g in range(CHUNK // GROUP):
                    base = g * GROUP
                    ps = ppool.tile([128, 2 * GROUP], F32)
                    xs1 = xt[:, base : base + 512]
                    xs2 = xt[:, base + 512 : base + 1024]
                    # u = ps[:, 0:1024], w = ps[:, 1024:2048]
                    nc.tensor.matmul(ps[:, 0:512], lhsT=mb, rhs=xs1,
                                     start=True, stop=True)
                    nc.tensor.matmul(ps[:, 512:1024], lhsT=mb, rhs=xs2,
                                     start=True, stop=True)
                    nc.tensor.matmul(ps[:, 1024:1536], lhsT=msw, rhs=xs1,
                                     start=True, stop=True)
                    nc.tensor.matmul(ps[:, 1536:2048], lhsT=msw, rhs=xs2,
                                     start=True, stop=True)

                    t1 = tpool.tile([128, GROUP], BF16)
                    t2 = tpool.tile([128, GROUP], BF16)
                    uw = tpool.tile([128, 1536], BF16)
                    # DVE drain-mul of u1 (PSUM 1x)
                    nc.vector.tensor_mul(
                        out=t1[:, 0:512], in0=ps[:, 0:512], in1=ctab[:, 0:512]
                    )
                    # Scalar engine drains u2 + w in one ACT copy+cast
                    nc.scalar.copy(out=uw, in_=ps[:, 512:2048])
                    # bf16 2x muls on DVE
                    nc.vector.tensor_mul(
                        out=t1[:, 512:1024], in0=uw[:, 0:512],
                        in1=ctab[:, 512:1024]
                    )
                    nc.vector.tensor_mul(out=t2, in0=uw[:, 512:1536], in1=stab)
                    # final add: every 4th group on DVE, rest on GpSimd
                    ov = ot[:, base : base + GROUP]
                    if g % DVE_ADD_EVERY == DVE_ADD_EVERY - 1:
                        nc.vector.tensor_add(out=ov, in0=t1, in1=t2)
                    else:
                        nc.gpsimd.tensor_add(out=ov, in0=t1, in1=t2)
                nc.sync.dma_start(
                    out=o_d.ap()[:, ch * CHUNK : (ch + 1) * CHUNK], in_=ot
                )
    _split_excess_waits(nc)
    _NC_CACHE["nc"] = nc
    return nc


def kernel(x, thetas, rotation_matrix, inv_freq, _trace=False):
    from concourse.bass_utils import run_bass_kernel_spmd

    x = np.asarray(x, dtype=np.float32)
    thetas = np.asarray(thetas, dtype=np.float32)
    rotation_matrix = np.asarray(rotation_matrix, dtype=np.float32)
    inv_freq = np.asarray(inv_freq, dtype=np.float32)

    Mbig, Msw = build_weights(thetas, rotation_matrix)
    ctabs, stabs = build_tables(inv_freq)
    xs = shard_x(x)

    nc = _build_nc()
    in_maps = [
        {"x": xs[c], "mb": Mbig, "msw": Msw, "ctab": ctabs[c], "stab": stabs[c]}
        for c in range(NCORES)
    ]
    res = run_bass_kernel_spmd(
        nc, in_maps, core_ids=list(range(NCORES)), trace=_trace
    )
    o = np.stack([res.results[c]["o"] for c in range(NCORES)])
    out = unshard_out(o)
    if _trace:
        return out, res
    return out


# revision 7
# speedup vs baseline: 1.3557x; 1.2580x over previous
"""Trainium2 Bass kernel for nn_CombinedRotaryEmbedding.

Math: the 32 sequential Givens rotations and the learned rotation_matrix
compose into a single 64x64 matrix M (host-precomputed).  The RoPE stage
  out_top = y1*cos - y2*sin ; out_bot = y1*sin + y2*cos
is rewritten as out = u # COS + w # SIN with
  u = x @ Mbig   (rows = [Y1 | Y2] per head-pair)
  w = x @ Msw    (rows = [-Y2 | Y1])
so no cross-partition data movement is needed on-device.

v2: all HBM I/O in bf16 (halves the DMA roofline to ~45us/core), bf16
matmuls (4x PE rate vs fp32r).  The fp32-PSUM drain is the critical
resource: DVE tensor_tensor from PSUM runs at 1x (no 16-bit packing), so
the drain is split across engines per group of 1024 columns:
  - DVE drain-mul: t1[0:512]   = u1 # cos      (PSUM, 1x, mul fused free)
  - Scalar casts:  ub2 = bf16(u2), wb = bf16(w) (ACT copy, idle engine)
  - DVE bf16 2x:   t1[512:1024] = ub2 # cos ; t2 = wb # sin
  - adds t1+t2 split GpSimd (0:768) / DVE (768:1024)

Sharding: sequence-parallel over 8 cores (1024 positions each).  The host
pre-transposes x to [core][128 partitions = (head%2, d_in)][b, head//2, s]
so the PE can contract over d_in with full 128-partition utilisation, and
inverse-permutes the output.
"""

import numpy as np


def _import_bass():
    try:
        import concourse.bass  # noqa: F401
    except ImportError:
        import sys

        sys.path.insert(0, "/opt/trn_rl_repo")


_import_bass()

import concourse.bass as bass  # noqa: E402
import concourse.mybir as mybir  # noqa: E402
from concourse.tile import TileContext  # noqa: E402
from concourse.vector_clock import ScopedClock  # noqa: E402

import ml_dtypes  # noqa: E402

B, S, NSTATE = 4, 8192, 1024
H, D, NUM_ROT = 16, 64, 32
NCORES = 8
S_SH = S // NCORES  # 1024 positions per core
FREE = B * (H // 2) * S_SH  # 32768 columns per core
CHUNK = 8192  # x-columns per DMA (16KB bf16 per partition line)
GROUP = 1024  # x-columns per PSUM group == one (b, head-pair) s-block
DVE_DRAIN = 192  # u-cols the DVE drains from PSUM; Scalar casts the rest

F32 = mybir.dt.float32
BF16 = mybir.dt.bfloat16
NP_BF16 = ml_dtypes.bfloat16


class _TileContextSplitDrain(TileContext):
    """TileContext whose final drain carries at most one sem wait per
    instruction — the walrus in this container rejects instructions
    with 2+ sync waits ("Too many sync wait commands")."""

    def _drain_and_barrier(self, tick_clock, wait_clock):
        nc = self.nc
        drain_inst = nc.sync.drain()
        wait_clock.add_sem_waits(
            drain_inst.ins, ScopedClock({None: tick_clock.global_clock})
        )
        si = drain_inst.ins.sync_info
        waits = list(si.on_wait or [])
        if len(waits) > 1:
            si.on_wait = [waits[0]]
            for w in waits[1:]:
                n = nc.sync.nop(nofuse=True, hint="drain_wait_split")
                n.ins.sync_info = type(si)(on_update=[], on_wait=[w])
        nc.all_engine_barrier()
        assert self.sems is not None
        popped = nc._tile_sem_poison_stack.pop()
        assert popped is self._sem_poison
        nc.clear_and_free_semaphores(list(self.sems.allocated().values()))
        nc.all_engine_barrier()


def _split_excess_waits(nc, limit=1):
    """Walrus here rejects instructions with >limit sync waits.  Hoist
    excess waits onto same-engine InstNoOps inserted immediately before
    the offending instruction (same engine stream => program order)."""
    n_split = 0
    for fn in nc.m.functions:
        for blk in fn.blocks:
            insts = blk.instructions
            i = 0
            while i < len(insts):
                inst = insts[i]
                si = getattr(inst, "sync_info", None)
                waits = list(si.on_wait) if (si and si.on_wait) else []
                if len(waits) > limit:
                    keep = waits[-limit:]
                    excess = waits[:-limit]
                    si.on_wait = keep
                    for j, w in enumerate(excess):
                        nop = mybir.InstNoOp(
                            name=f"{inst.name}-wsplit{j}",
                            engine=inst.engine,
                            bass_nofuse=True,
                            sync_info=mybir.SyncInfo(on_wait=[w], on_update=[]),
                        )
                        insts.insert(i, nop)
                        i += 1
                        n_split += 1
                i += 1
    return n_split


def compose_rotation(thetas: np.ndarray, rotation_matrix: np.ndarray) -> np.ndarray:
    """Fold the sequential Givens rotations + rotation_matrix into one 64x64."""
    M = np.eye(D, dtype=np.float64)
    th = thetas.astype(np.float64)
    for k in range(NUM_ROT):
        i, j = k % D, (k + 1) % D
        c, s = np.cos(th[k]), np.sin(th[k])
        mi = M[:, i] * c + M[:, j] * s
        mj = -M[:, i] * s + M[:, j] * c
        M[:, i], M[:, j] = mi, mj
    return M @ rotation_matrix.astype(np.float64)


def build_weights(thetas: np.ndarray, rotation_matrix: np.ndarray):
    """Mbig (u = [Y1|Y2]) and Msw (w = [-Y2|Y1]) as [k=128, m=128] bf16."""
    M64 = compose_rotation(thetas, rotation_matrix)
    Mev = M64[:, 0::2]  # y1 columns [64, 32]
    Mod = M64[:, 1::2]  # y2 columns
    Mbig = np.zeros((128, 128), dtype=np.float64)
    Msw = np.zeros((128, 128), dtype=np.float64)
    for hp in (0, 1):
        r = slice(hp * 64, hp * 64 + 64)
        c1 = slice(hp * 32, hp * 32 + 32)
        c2 = slice(64 + hp * 32, 64 + hp * 32 + 32)
        Mbig[r, c1] = Mev
        Mbig[r, c2] = Mod
        Msw[r, c1] = -Mod
        Msw[r, c2] = Mev
    return Mbig.astype(NP_BF16), Msw.astype(NP_BF16)


def build_tables(inv_freq: np.ndarray):
    """Per-core cos/sin tables [128, 1024] each, row p uses inv_freq[p % 32],
    column j is position core_base + j.

    Args are computed in fp32 to match the reference's fp32 `pos * inv_freq`
    rounding; sin/cos mirror the reference's jax lowering when available.
    """
    invf = inv_freq.astype(np.float32)
    try:
        import jax.numpy as jnp

        pos = jnp.arange(S, dtype=jnp.float32)
        sinusoid = pos[:, None] * jnp.asarray(invf)[None, :]  # [S, 32]
        sin_all = np.asarray(jnp.sin(sinusoid))
        cos_all = np.asarray(jnp.cos(sinusoid))
    except Exception:
        args = np.arange(S, dtype=np.float32)[:, None] * invf[None, :]
        sin_all, cos_all = np.sin(args), np.cos(args)

    l = np.arange(128) % 32
    ctabs = np.empty((NCORES, 128, GROUP), dtype=NP_BF16)
    stabs = np.empty((NCORES, 128, GROUP), dtype=NP_BF16)
    for c in range(NCORES):
        sl = slice(c * S_SH, (c + 1) * S_SH)
        ctabs[c] = cos_all[sl].T[l].astype(NP_BF16)  # [128, 1024]
        stabs[c] = sin_all[sl].T[l].astype(NP_BF16)
    return ctabs, stabs


def shard_x(x: np.ndarray) -> np.ndarray:
    """[B,S,1024] -> [core, 128 (hp,d), FREE (b,hi,s)] contiguous bf16."""
    xr = np.ascontiguousarray(x).reshape(B, NCORES, S_SH, H // 2, 2, D)
    xt = xr.transpose(1, 4, 5, 0, 3, 2)  # (core, hp, d, b, hi, sl)
    return np.ascontiguousarray(xt).astype(NP_BF16).reshape(NCORES, 128, FREE)


def unshard_out(o: np.ndarray) -> np.ndarray:
    """[core, 128 (half,hp,l), FREE (b,hi,s)] bf16 -> [B,S,1024] f32."""
    orr = o.astype(np.float32).reshape(NCORES, 2, 2, 32, B, H // 2, S_SH)
    ot = orr.transpose(4, 0, 6, 5, 2, 1, 3)  # (b, core, sl, hi, hp, half, l)
    return np.ascontiguousarray(ot).reshape(B, S, NSTATE)


_NC_CACHE = {}


def _build_nc():
    if "nc" in _NC_CACHE:
        return _NC_CACHE["nc"]
    nc = bass.Bass(trn_type="TRN2")
    x_d = nc.dram_tensor("x", [128, FREE], BF16, kind="ExternalInput")
    mb_d = nc.dram_tensor("mb", [128, 128], BF16, kind="ExternalInput")
    msw_d = nc.dram_tensor("msw", [128, 128], BF16, kind="ExternalInput")
    ctab_d = nc.dram_tensor("ctab", [128, GROUP], BF16, kind="ExternalInput")
    stab_d = nc.dram_tensor("stab", [128, GROUP], BF16, kind="ExternalInput")
    o_d = nc.dram_tensor("o", [128, FREE], BF16, kind="ExternalOutput")

    with _TileContextSplitDrain(nc) as tc:
        with tc.tile_pool(name="const", bufs=1) as cpool, \
             tc.tile_pool(name="xin", bufs=3) as xpool, \
             tc.tile_pool(name="t12", bufs=3) as tpool, \
             tc.tile_pool(name="oout", bufs=2) as opool, \
             tc.tile_pool(name="psum", bufs=2, space="PSUM") as ppool:
            mb = cpool.tile([128, 128], BF16, tag="mb")
            msw = cpool.tile([128, 128], BF16, tag="msw")
            ctab = cpool.tile([128, GROUP], BF16, tag="ctab")
            stab = cpool.tile([128, GROUP], BF16, tag="stab")
            nc.sync.dma_start(out=mb, in_=mb_d.ap())
            nc.sync.dma_start(out=msw, in_=msw_d.ap())
            nc.sync.dma_start(out=ctab, in_=ctab_d.ap())
            nc.sync.dma_start(out=stab, in_=stab_d.ap())

            for ch in range(FREE // CHUNK):
                xt = xpool.tile([128, CHUNK], BF16)
                nc.sync.dma_start(
                    out=xt, in_=x_d.ap()[:, ch * CHUNK : (ch + 1) * CHUNK]
                )
                ot = opool.tile([128, CHUNK], BF16)
                for g in range(CHUNK // GROUP):
                    base = g * GROUP
                    ps = ppool.tile([128, 2 * GROUP], F32)
                    xs1 = xt[:, base : base + 512]
                    xs2 = xt[:, base + 512 : base + 1024]
                    # u = ps[:, 0:1024], w = ps[:, 1024:2048]
                    nc.tensor.matmul(ps[:, 0:512], lhsT=mb, rhs=xs1,
                                     start=True, stop=True)
                    nc.tensor.matmul(ps[:, 512:1024], lhsT=mb, rhs=xs2,
                                     start=True, stop=True)
                    nc.tensor.matmul(ps[:, 1024:1536], lhsT=msw, rhs=xs1,
                                     start=True, stop=True)
                    nc.tensor.matmul(ps[:, 1536:2048], lhsT=msw, rhs=xs2,
                                     start=True, stop=True)

                    t1 = tpool.tile([128, GROUP], BF16)
                    t2 = tpool.tile([128, GROUP], BF16)
                    uw = tpool.tile([128, 2048 - DVE_DRAIN], BF16)
                    # DVE drain-mul of the first DVE_DRAIN u-cols (PSUM 1x).
                    # Deliberately small: the Scalar engine is the faster
                    # PSUM drain, and GpSimd is unusable here (its SBUF port
                    # is shared with the DVE and wrecks the 2x bf16 ops).
                    nc.vector.tensor_mul(
                        out=t1[:, 0:DVE_DRAIN],
                        in0=ps[:, 0:DVE_DRAIN],
                        in1=ctab[:, 0:DVE_DRAIN],
                    )
                    # Scalar engine drains the rest of u + all of w
                    nc.scalar.copy(out=uw, in_=ps[:, DVE_DRAIN:2048])
                    # bf16 2x muls + add on DVE
                    nc.vector.tensor_mul(
                        out=t1[:, DVE_DRAIN:GROUP],
                        in0=uw[:, 0 : GROUP - DVE_DRAIN],
                        in1=ctab[:, DVE_DRAIN:GROUP],
                    )
                    nc.vector.tensor_mul(
                        out=t2, in0=uw[:, GROUP - DVE_DRAIN : 2048 - DVE_DRAIN],
                        in1=stab,
                    )
                    ov = ot[:, base : base + GROUP]
                    nc.vector.tensor_add(out=ov, in0=t1, in1=t2)
                nc.sync.dma_start(
                    out=o_d.ap()[:, ch * CHUNK : (ch + 1) * CHUNK], in_=ot
                )
    _split_excess_waits(nc)
    _NC_CACHE["nc"] = nc
    return nc


def kernel(x, thetas, rotation_matrix, inv_freq, _trace=False):
    from concourse.bass_utils import run_bass_kernel_spmd

    x = np.asarray(x, dtype=np.float32)
    thetas = np.asarray(thetas, dtype=np.float32)
    rotation_matrix = np.asarray(rotation_matrix, dtype=np.float32)
    inv_freq = np.asarray(inv_freq, dtype=np.float32)

    Mbig, Msw = build_weights(thetas, rotation_matrix)
    ctabs, stabs = build_tables(inv_freq)
    xs = shard_x(x)

    nc = _build_nc()
    in_maps = [
        {"x": xs[c], "mb": Mbig, "msw": Msw, "ctab": ctabs[c], "stab": stabs[c]}
        for c in range(NCORES)
    ]
    res = run_bass_kernel_spmd(
        nc, in_maps, core_ids=list(range(NCORES)), trace=_trace
    )
    o = np.stack([res.results[c]["o"] for c in range(NCORES)])
    out = unshard_out(o)
    if _trace:
        return out, res
    return out


# revision 8
# speedup vs baseline: 1.3656x; 1.0073x over previous
"""Trainium2 Bass kernel for nn_CombinedRotaryEmbedding.

Math: the 32 sequential Givens rotations and the learned rotation_matrix
compose into a single 64x64 matrix M (host-precomputed).  The RoPE stage
  out_top = y1*cos - y2*sin ; out_bot = y1*sin + y2*cos
is rewritten as out = u # COS + w # SIN with
  u = x @ Mbig   (rows = [Y1 | Y2] per head-pair)
  w = x @ Msw    (rows = [-Y2 | Y1])
so no cross-partition data movement is needed on-device.

v2: all HBM I/O in bf16 (halves the DMA roofline to ~45us/core), bf16
matmuls (4x PE rate vs fp32r).  The fp32-PSUM drain is the critical
resource: DVE tensor_tensor from PSUM runs at 1x (no 16-bit packing), so
the drain is split across engines per group of 1024 columns:
  - DVE drain-mul: t1[0:512]   = u1 # cos      (PSUM, 1x, mul fused free)
  - Scalar casts:  ub2 = bf16(u2), wb = bf16(w) (ACT copy, idle engine)
  - DVE bf16 2x:   t1[512:1024] = ub2 # cos ; t2 = wb # sin
  - adds t1+t2 split GpSimd (0:768) / DVE (768:1024)

Sharding: sequence-parallel over 8 cores (1024 positions each).  The host
pre-transposes x to [core][128 partitions = (head%2, d_in)][b, head//2, s]
so the PE can contract over d_in with full 128-partition utilisation, and
inverse-permutes the output.
"""

import numpy as np


def _import_bass():
    try:
        import concourse.bass  # noqa: F401
    except ImportError:
        import sys

        sys.path.insert(0, "/opt/trn_rl_repo")


_import_bass()

import concourse.bass as bass  # noqa: E402
import concourse.mybir as mybir  # noqa: E402
from concourse.tile import TileContext  # noqa: E402
from concourse.vector_clock import ScopedClock  # noqa: E402

import ml_dtypes  # noqa: E402

B, S, NSTATE = 4, 8192, 1024
H, D, NUM_ROT = 16, 64, 32
NCORES = 8
S_SH = S // NCORES  # 1024 positions per core
FREE = B * (H // 2) * S_SH  # 32768 columns per core
CHUNK = 8192  # x-columns per DMA (16KB bf16 per partition line)
GROUP = 1024  # x-columns per PSUM group == one (b, head-pair) s-block
DVE_DRAIN = 192  # u-cols the DVE drains from PSUM; Scalar casts the rest

F32 = mybir.dt.float32
BF16 = mybir.dt.bfloat16
NP_BF16 = ml_dtypes.bfloat16


class _TileContextSplitDrain(TileContext):
    """TileContext whose final drain carries at most one sem wait per
    instruction — the walrus in this container rejects instructions
    with 2+ sync waits ("Too many sync wait commands")."""

    def _drain_and_barrier(self, tick_clock, wait_clock):
        nc = self.nc
        drain_inst = nc.sync.drain()
        wait_clock.add_sem_waits(
            drain_inst.ins, ScopedClock({None: tick_clock.global_clock})
        )
        si = drain_inst.ins.sync_info
        waits = list(si.on_wait or [])
        if len(waits) > 1:
            si.on_wait = [waits[0]]
            for w in waits[1:]:
                n = nc.sync.nop(nofuse=True, hint="drain_wait_split")
                n.ins.sync_info = type(si)(on_update=[], on_wait=[w])
        nc.all_engine_barrier()
        assert self.sems is not None
        popped = nc._tile_sem_poison_stack.pop()
        assert popped is self._sem_poison
        nc.clear_and_free_semaphores(list(self.sems.allocated().values()))
        nc.all_engine_barrier()


def _split_excess_waits(nc, limit=1):
    """Walrus here rejects instructions with >limit sync waits.  Hoist
    excess waits onto same-engine InstNoOps inserted immediately before
    the offending instruction (same engine stream => program order)."""
    n_split = 0
    for fn in nc.m.functions:
        for blk in fn.blocks:
            insts = blk.instructions
            i = 0
            while i < len(insts):
                inst = insts[i]
                si = getattr(inst, "sync_info", None)
                waits = list(si.on_wait) if (si and si.on_wait) else []
                if len(waits) > limit:
                    keep = waits[-limit:]
                    excess = waits[:-limit]
                    si.on_wait = keep
                    for j, w in enumerate(excess):
                        nop = mybir.InstNoOp(
                            name=f"{inst.name}-wsplit{j}",
                            engine=inst.engine,
                            bass_nofuse=True,
                            sync_info=mybir.SyncInfo(on_wait=[w], on_update=[]),
                        )
                        insts.insert(i, nop)
                        i += 1
                        n_split += 1
                i += 1
    return n_split


def compose_rotation(thetas: np.ndarray, rotation_matrix: np.ndarray) -> np.ndarray:
    """Fold the sequential Givens rotations + rotation_matrix into one 64x64."""
    M = np.eye(D, dtype=np.float64)
    th = thetas.astype(np.float64)
    for k in range(NUM_ROT):
        i, j = k % D, (k + 1) % D
        c, s = np.cos(th[k]), np.sin(th[k])
        mi = M[:, i] * c + M[:, j] * s
        mj = -M[:, i] * s + M[:, j] * c
        M[:, i], M[:, j] = mi, mj
    return M @ rotation_matrix.astype(np.float64)


def build_weights(thetas: np.ndarray, rotation_matrix: np.ndarray):
    """Mbig (u = [Y1|Y2]) and Msw (w = [-Y2|Y1]) as [k=128, m=128] bf16."""
    M64 = compose_rotation(thetas, rotation_matrix)
    Mev = M64[:, 0::2]  # y1 columns [64, 32]
    Mod = M64[:, 1::2]  # y2 columns
    Mbig = np.zeros((128, 128), dtype=np.float64)
    Msw = np.zeros((128, 128), dtype=np.float64)
    for hp in (0, 1):
        r = slice(hp * 64, hp * 64 + 64)
        c1 = slice(hp * 32, hp * 32 + 32)
        c2 = slice(64 + hp * 32, 64 + hp * 32 + 32)
        Mbig[r, c1] = Mev
        Mbig[r, c2] = Mod
        Msw[r, c1] = -Mod
        Msw[r, c2] = Mev
    return Mbig.astype(NP_BF16), Msw.astype(NP_BF16)


def build_tables(inv_freq: np.ndarray):
    """Per-core cos/sin tables [128, 1024] each, row p uses inv_freq[p % 32],
    column j is position core_base + j.

    Args are computed in fp32 to match the reference's fp32 `pos * inv_freq`
    rounding; sin/cos mirror the reference's jax lowering when available.
    """
    invf = inv_freq.astype(np.float32)
    try:
        import jax.numpy as jnp

        pos = jnp.arange(S, dtype=jnp.float32)
        sinusoid = pos[:, None] * jnp.asarray(invf)[None, :]  # [S, 32]
        sin_all = np.asarray(jnp.sin(sinusoid))
        cos_all = np.asarray(jnp.cos(sinusoid))
    except Exception:
        args = np.arange(S, dtype=np.float32)[:, None] * invf[None, :]
        sin_all, cos_all = np.sin(args), np.cos(args)

    l = np.arange(128) % 32
    ctabs = np.empty((NCORES, 128, GROUP), dtype=NP_BF16)
    stabs = np.empty((NCORES, 128, GROUP), dtype=NP_BF16)
    for c in range(NCORES):
        sl = slice(c * S_SH, (c + 1) * S_SH)
        ctabs[c] = cos_all[sl].T[l].astype(NP_BF16)  # [128, 1024]
        stabs[c] = sin_all[sl].T[l].astype(NP_BF16)
    return ctabs, stabs


def shard_x(x: np.ndarray) -> np.ndarray:
    """[B,S,1024] -> [core, 128 (hp,d), FREE (b,hi,s)] contiguous bf16."""
    xr = np.ascontiguousarray(x).reshape(B, NCORES, S_SH, H // 2, 2, D)
    xt = xr.transpose(1, 4, 5, 0, 3, 2)  # (core, hp, d, b, hi, sl)
    return np.ascontiguousarray(xt).astype(NP_BF16).reshape(NCORES, 128, FREE)


def unshard_out(o: np.ndarray) -> np.ndarray:
    """[core, 128 (half,hp,l), FREE (b,hi,s)] bf16 -> [B,S,1024] f32."""
    orr = o.astype(np.float32).reshape(NCORES, 2, 2, 32, B, H // 2, S_SH)
    ot = orr.transpose(4, 0, 6, 5, 2, 1, 3)  # (b, core, sl, hi, hp, half, l)
    return np.ascontiguousarray(ot).reshape(B, S, NSTATE)


_NC_CACHE = {}


def _build_nc():
    if "nc" in _NC_CACHE:
        return _NC_CACHE["nc"]
    nc = bass.Bass(trn_type="TRN2")
    x_d = nc.dram_tensor("x", [128, FREE], BF16, kind="ExternalInput")
    mb_d = nc.dram_tensor("mb", [128, 128], BF16, kind="ExternalInput")
    msw_d = nc.dram_tensor("msw", [128, 128], BF16, kind="ExternalInput")
    ctab_d = nc.dram_tensor("ctab", [128, GROUP], BF16, kind="ExternalInput")
    stab_d = nc.dram_tensor("stab", [128, GROUP], BF16, kind="ExternalInput")
    o_d = nc.dram_tensor("o", [128, FREE], BF16, kind="ExternalOutput")

    with _TileContextSplitDrain(nc) as tc:
        with tc.tile_pool(name="const", bufs=1) as cpool, \
             tc.tile_pool(name="xin", bufs=3) as xpool, \
             tc.tile_pool(name="t12", bufs=3) as tpool, \
             tc.tile_pool(name="oout", bufs=2) as opool, \
             tc.tile_pool(name="psum", bufs=2, space="PSUM") as ppool:
            mb = cpool.tile([128, 128], BF16, tag="mb")
            msw = cpool.tile([128, 128], BF16, tag="msw")
            ctab = cpool.tile([128, GROUP], BF16, tag="ctab")
            stab = cpool.tile([128, GROUP], BF16, tag="stab")
            nc.sync.dma_start(out=mb, in_=mb_d.ap())
            nc.sync.dma_start(out=msw, in_=msw_d.ap())
            nc.sync.dma_start(out=ctab, in_=ctab_d.ap())
            nc.sync.dma_start(out=stab, in_=stab_d.ap())

            for ch in range(FREE // CHUNK):
                xt = xpool.tile([128, CHUNK], BF16)
                nc.sync.dma_start(
                    out=xt, in_=x_d.ap()[:, ch * CHUNK : (ch + 1) * CHUNK]
                )
                ot = opool.tile([128, CHUNK], BF16)
                for g in range(CHUNK // GROUP):
                    base = g * GROUP
                    ps = ppool.tile([128, 2 * GROUP], F32)
                    xs1 = xt[:, base : base + 512]
                    xs2 = xt[:, base + 512 : base + 1024]
                    # u = ps[:, 0:1024], w = ps[:, 1024:2048]
                    nc.tensor.matmul(ps[:, 0:512], lhsT=mb, rhs=xs1,
                                     start=True, stop=True)
                    nc.tensor.matmul(ps[:, 512:1024], lhsT=mb, rhs=xs2,
                                     start=True, stop=True)
                    nc.tensor.matmul(ps[:, 1024:1536], lhsT=msw, rhs=xs1,
                                     start=True, stop=True)
                    nc.tensor.matmul(ps[:, 1536:2048], lhsT=msw, rhs=xs2,
                                     start=True, stop=True)

                    t1 = tpool.tile([128, GROUP], BF16)
                    t2 = tpool.tile([128, GROUP], BF16)
                    uw = tpool.tile([128, 2048], BF16)
                    # Scalar engine drains the whole PSUM group (it is the
                    # fastest PSUM drain and has its own ports; GpSimd is
                    # unusable here -- its SBUF port is shared with the DVE
                    # and wrecks the 2x bf16 ops).
                    nc.scalar.copy(out=uw, in_=ps)
                    # bf16 2x muls + add on DVE
                    nc.vector.tensor_mul(
                        out=t1, in0=uw[:, 0:GROUP], in1=ctab
                    )
                    nc.vector.tensor_mul(
                        out=t2, in0=uw[:, GROUP:2048], in1=stab
                    )
                    ov = ot[:, base : base + GROUP]
                    nc.vector.tensor_add(out=ov, in0=t1, in1=t2)
                nc.sync.dma_start(
                    out=o_d.ap()[:, ch * CHUNK : (ch + 1) * CHUNK], in_=ot
                )
    _split_excess_waits(nc)
    _NC_CACHE["nc"] = nc
    return nc


def kernel(x, thetas, rotation_matrix, inv_freq, _trace=False):
    from concourse.bass_utils import run_bass_kernel_spmd

    x = np.asarray(x, dtype=np.float32)
    thetas = np.asarray(thetas, dtype=np.float32)
    rotation_matrix = np.asarray(rotation_matrix, dtype=np.float32)
    inv_freq = np.asarray(inv_freq, dtype=np.float32)

    Mbig, Msw = build_weights(thetas, rotation_matrix)
    ctabs, stabs = build_tables(inv_freq)
    xs = shard_x(x)

    nc = _build_nc()
    in_maps = [
        {"x": xs[c], "mb": Mbig, "msw": Msw, "ctab": ctabs[c], "stab": stabs[c]}
        for c in range(NCORES)
    ]
    res = run_bass_kernel_spmd(
        nc, in_maps, core_ids=list(range(NCORES)), trace=_trace
    )
    o = np.stack([res.results[c]["o"] for c in range(NCORES)])
    out = unshard_out(o)
    if _trace:
        return out, res
    return out
